# revision 53
# baseline (speedup 1.0000x reference)
"""Trainium2 Bass kernel for nn_Attention_model_44057774522458.

Bahdanau-style attention:
    h = hidden @ W2 + W2_b                       [B, U]
    score = tanh(features @ W1 + W1_b + h[:,None,:])   [B, L, U]
    logits = score @ V + V_b                     [B, L, 1]
    attn = softmax(logits, axis=1)               [B, L, 1]
    context = sum(attn * features, axis=1)       [B, D]
    returns (context, attn)

B=512, L=64, D=1024, U=512. Data-parallel over B across 8 NeuronCores
(64 batches/core). Per core, batches are processed in 32 groups of 2
(2*L = 128 PE partitions).

Per-group pipeline on each core:
  PE:  score psum [128,512] = sum_c featT[c].T @ W1[c]  (8 bf16 matmuls,
       K=128 each) + indicator.T @ g  (K=2 matmul adds the per-batch
       h+bias correction broadcast over L via a 2-row indicator matrix)
  ACT: tanh psum -> sbuf bf16
  DVE: tensor_tensor_reduce with V broadcast -> logits column [128,1]
Softmax is batched once per core on a [32,128] transpose of the staged
logits ([groups, 2*L] layout: reductions land on the free dim), and
context is a second PE pass: block-diagonal attn columns as stationary
operand against the naturally-laid-out features, accumulating all 32
groups into one [64,512] psum bank per 512-wide D chunk.

features are shipped in two host-packed bf16 layouts (transposed for the
score matmul, natural for the context matmul), 1 MiB contiguous per
4-group superblock per layout.
"""

import numpy as np
import ml_dtypes

import concourse.bass as bass
import concourse.tile as tile
from concourse import mybir
from concourse.bass_utils import run_bass_kernel_spmd

# ---------------------------------------------------------------------------
# Workaround for walrus "Too many sync wait commands" on the TileContext tail
# drain: this neuronxcc build accepts only a single sync-wait on the Drain
# instruction, while Tile attaches one wait per live semaphore. Emit one
# standalone SP wait per (sem, final tick) from the global vector clock
# (validated to reproduce add_sem_waits' wait set exactly), then a wait-free
# drain, then the usual barrier + semaphore clear.
# ---------------------------------------------------------------------------


def _drain_and_barrier_split(self, tick_clock, wait_clock):
    nc = self.nc
    gc = list(tick_clock.global_clock)
    alloc = self.sems.allocated()
    for proc, sem in sorted(alloc.items()):
        tick = gc[proc]
        if tick <= 0:
            continue
        scale = 16 if sem.name.startswith("DMA") else 1
        nc.sync.wait_ge(sem, tick * scale)
    nc.sync.drain()
    nc.all_engine_barrier()
    assert self.sems is not None
    popped = nc._tile_sem_poison_stack.pop()
    assert popped is self._sem_poison
    nc.clear_and_free_semaphores(list(self.sems.allocated().values()))
    nc.all_engine_barrier()


tile.TileContext._drain_and_barrier = _drain_and_barrier_split

# This walrus build also rejects >1 sync-wait on at least the Drain and
# LDWEIGHTS instruction structs ("Too many sync wait commands"). Peel every
# wait beyond the first off onto dedicated InstEventSemaphore instructions
# committed immediately before, on the same engine — engines execute their
# stream in order, so a wait on the preceding instruction is equivalent.
_MAX_WAITS = 1
_orig_commit = tile.TileContext._commit_instruction


def _commit_split_waits(self, inst, lazy_reg_writes=True):
    import bass_rust as _br

    si = inst.sync_info
    if si is not None and len(si.on_wait) > _MAX_WAITS:
        waits = list(si.on_wait)
        keep, extra = waits[:_MAX_WAITS], waits[_MAX_WAITS:]
        for w in extra:
            ev = mybir.InstEventSemaphore(
                name=self.nc.get_next_instruction_name(), ins=[], outs=[]
            )
            ev.engine = inst.engine
            ev.sync_info = _br.SyncInfo(on_wait=[w], on_update=[])
            _orig_commit(self, ev, lazy_reg_writes)
        inst.sync_info = _br.SyncInfo(on_wait=keep, on_update=list(si.on_update))
    return _orig_commit(self, inst, lazy_reg_writes)


tile.TileContext._commit_instruction = _commit_split_waits

BF16 = mybir.dt.bfloat16
FP32 = mybir.dt.float32
FP32R = mybir.dt.float32r  # fp32 bits, full-rate PE mode (1 cyc/row at N>=256)
NP_BF16 = ml_dtypes.bfloat16

B, L, D, U = 512, 64, 1024, 512
NCORES = 8
BC = B // NCORES          # batches per core = 64
G = BC // 2               # 2-batch groups per core = 32
NS = G // 4               # 4-group superblocks per core = 8
DC = D // 128             # 1024/128 = 8 contraction chunks
UC = U // 128             # 512/128 = 4 chunks (h matmul contraction)

_NC_CACHE = None


def _build_program():
    nc = bass.Bass()
    dp = nc.declare_dram_parameter
    featT_d = dp("featT", [NS, 128, 4 * D], BF16, isOutput=False)
    nat_d = dp("nat", [NS, 128, 4 * D], BF16, isOutput=False)
    hT_d = dp("hT", [128, UC, BC], BF16, isOutput=False)
    w1_d = dp("w1", [128, DC, U], BF16, isOutput=False)
    w2_d = dp("w2", [128, UC, U], BF16, isOutput=False)
    vbc_d = dp("vbc", [128, U], BF16, isOutput=False)
    ident_d = dp("ident", [128, 128], FP32, isOutput=False)
    ind_d = dp("ind64", [128, G, 128], BF16, isOutput=False)
    onesb_d = dp("onesb", [1, BC], BF16, isOutput=False)
    bsum_d = dp("bsum", [1, U], BF16, isOutput=False)
    ctx_d = dp("ctx", [BC, D], FP32, isOutput=True)
    attn_d = dp("attn", [BC, L], FP32, isOutput=True)

    Act = mybir.ActivationFunctionType
    Alu = mybir.AluOpType

    with tile.TileContext(nc) as tc:
        with (
            tc.tile_pool(name="consts", bufs=1) as cp,
            tc.tile_pool(name="natp", bufs=1) as natp,
            tc.tile_pool(name="featp", bufs=7) as featp,
            tc.tile_pool(name="work", bufs=3) as wp,
            tc.tile_pool(name="ttrp", bufs=2) as ttrp,
            tc.tile_pool(name="psmain", bufs=4, space="PSUM") as psm,
            tc.tile_pool(name="psctx", bufs=1, space="PSUM") as psc,
            tc.tile_pool(name="pst", bufs=1, space="PSUM") as pst,
        ):
            # ---- loads, ordered by when the PE first needs them ----
            # featT is fetched per group (256 KiB each) with a 6-group
            # prefetch window so the PE never waits; nat loads (needed only
            # in phase D) are spread through the group loop.
            featT_tiles = {}

            def fetch_featT(g):
                s, gl = divmod(g, 4)
                t = featp.tile([128, D], BF16, tag="featT")
                nc.sync.dma_start(t[:], featT_d[s][:, gl * D:(gl + 1) * D])
                featT_tiles[g] = t

            fetch_featT(0)
            w1_t = cp.tile([128, DC, U], BF16, tag="w1")
            nc.sync.dma_start(w1_t[:, 0, :], w1_d[:, 0, :])
            fetch_featT(1)
            fetch_featT(2)
            nc.sync.dma_start(w1_t[:, 1:4, :], w1_d[:, 1:4, :])
            nc.sync.dma_start(w1_t[:, 4:, :], w1_d[:, 4:, :])
            hT_t = cp.tile([128, UC, BC], BF16, tag="hT")
            nc.sync.dma_start(hT_t[:], hT_d[:])
            w2_t = cp.tile([128, UC, U], BF16, tag="w2")
            nc.sync.dma_start(w2_t[:], w2_d[:])
            for g in range(3, 6):
                fetch_featT(g)
            # late-needed consts dispatch in parallel on the Scalar HWDGE ring
            vbc_t = cp.tile([128, U], BF16, tag="vbc")
            nc.scalar.dma_start(vbc_t[:], vbc_d[:])
            ind_t = cp.tile([128, G, 128], BF16, tag="ind64")
            nc.scalar.dma_start(ind_t[:], ind_d[:])
            onesb_t = cp.tile([1, BC], BF16, tag="onesb")
            nc.scalar.dma_start(onesb_t[:], onesb_d[:])
            bsum_t = cp.tile([1, U], BF16, tag="bsum")
            nc.scalar.dma_start(bsum_t[:], bsum_d[:])
            ident_t = cp.tile([128, 128], FP32, tag="ident")
            nc.scalar.dma_start(ident_t[:], ident_d[:])

            zb = cp.tile([128, 1], FP32, tag="zb")
            nc.vector.memset(zb[:], 0.0)
            ones_col = cp.tile([128, 1], BF16, tag="ones_col")
            nc.vector.memset(ones_col[:], 1.0)
            # block-diagonal stationary operand for the context matmuls:
            # column 64*g + (2g+p) holds (unnormalized) exp-attn for batch
            # 2g+p on partitions p*64 .. p*64+64, all else zero.
            bd = cp.tile([128, G * BC], BF16, tag="bd")
            nc.vector.memset(bd[:], 0.0)

            nat_tiles = {}
            # g_bf is K=128 (rows 64-127 zero) so the indicator matmul's
            # weight load stays on the fast (FWL) path.
            g_bf = cp.tile([128, U], BF16, tag="g_bf")
            nc.vector.memset(g_bf[64:128, :], 0.0)
            logits = cp.tile([128, G], FP32, tag="logits")

            # ---- phase B: per-group score -> tanh -> V-dot ----
            # Groups 0-2 emit their feature matmuls first with the indicator
            # (h-broadcast) matmul deferred until after the g-chain, so the
            # PE streams on featT+W1 alone while hT/W2 are still in flight.
            NPRE = 3

            def score_mms(g, with_ind):
                # with_ind: the indicator (h-broadcast) matmul leads the
                # accumulation group so its weight load hides behind the
                # previous group's stream.
                featT_t = featT_tiles.pop(g)
                ps_s = psm.tile([128, U], FP32, tag="score")
                if with_ind:
                    nc.tensor.matmul(
                        ps_s[:], ind_t[:, g, :], g_bf[:], start=True, stop=False
                    )
                for c in range(DC):
                    nc.tensor.matmul(
                        ps_s[:], featT_t[:, c * 128:(c + 1) * 128], w1_t[:, c, :],
                        start=(c == 0 and not with_ind),
                        stop=(with_ind and c == DC - 1),
                    )
                return ps_s

            def finish_group(g, ps_s, emit_ind):
                if emit_ind:
                    nc.tensor.matmul(
                        ps_s[:], ind_t[:, g, :], g_bf[:], start=False, stop=True
                    )
                score_sb = wp.tile([128, U], BF16, tag="score_sb")
                nc.scalar.activation(score_sb[:], ps_s[:], Act.Tanh, bias=zb[:])
                ttr_out = ttrp.tile([128, U], BF16, tag="ttr")
                nc.vector.tensor_mul(ttr_out[:], score_sb[:], vbc_t[:])
                nc.vector.tensor_reduce(
                    out=logits[:, g:g + 1],
                    in_=ttr_out[:],
                    axis=mybir.AxisListType.X,
                    op=Alu.add,
                )

            exp_t = cp.tile([128, G], FP32, tag="exp_t")
            H = G // 2

            def exp_and_bd(half):
                # exp in logits layout + scatter into the block-diagonal
                # operand; context is normalized at the end instead (softmax
                # denominator folded into the psum->sbuf copy), keeping the
                # transpose + attn-output chain off the critical path.
                c0, c1 = half * H, (half + 1) * H
                nc.scalar.activation(
                    exp_t[:, c0:c1], logits[:, c0:c1], Act.Exp, bias=zb[:]
                )
                nc.vector.tensor_copy(
                    bd[0:64, 66 * c0:66 * (c1 - 1) + 1:66], exp_t[0:64, c0:c1]
                )
                nc.vector.tensor_copy(
                    bd[64:128, 66 * c0 + 1:66 * (c1 - 1) + 2:66],
                    exp_t[64:128, c0:c1],
                )

            nt = natp.tile([128, 4 * D], BF16, tag="nat0")
            nc.sync.dma_start(nt[:], nat_d[0])
            nat_tiles[0] = nt

            # prologue groups emit chunk-major so the PE consumes W1 chunks
            # and featT tiles in DMA arrival order without long stalls
            pre_feat = [featT_tiles.pop(g) for g in range(NPRE)]
            pre_psum = [
                psm.tile([128, U], FP32, tag="score", name=f"pre_ps{g}")
                for g in range(NPRE)
            ]
            for c in range(DC):
                for g in range(NPRE):
                    nc.tensor.matmul(
                        pre_psum[g][:],
                        pre_feat[g][:, c * 128:(c + 1) * 128],
                        w1_t[:, c, :],
                        start=(c == 0), stop=False,
                    )
            # g2 = hidden @ W2 + (W2_b + W1_b), fp32r full-rate
            ps_g = psm.tile([BC, U], FP32, tag="score")
            for c in range(UC):
                nc.tensor.matmul(
                    ps_g[:], hT_t[:, c, :], w2_t[:, c, :],
                    start=(c == 0), stop=False,
                )
            nc.tensor.matmul(ps_g[:], onesb_t[:], bsum_t[:], start=False, stop=True)
            nc.vector.tensor_copy(g_bf[0:BC, :], ps_g[:])
            for g in range(NPRE):
                fetch_featT(g + 6)
                finish_group(g, pre_psum[g], emit_ind=True)

            for g in range(NPRE, G):
                s, gl = divmod(g, 4)
                if gl == 0:
                    nt = natp.tile([128, 4 * D], BF16, tag=f"nat{s}")
                    nc.sync.dma_start(nt[:], nat_d[s])
                    nat_tiles[s] = nt
                if g + 6 < G:
                    fetch_featT(g + 6)
                with_ind = g > NPRE
                ps_s = score_mms(g, with_ind=with_ind)
                finish_group(g, ps_s, emit_ind=not with_ind)
                if g == H - 1:
                    exp_and_bd(0)
            exp_and_bd(1)

            # ---- phase D: context accumulation (unnormalized) ----
            # per-batch softmax denominators ride along in the ctx0 loop:
            # the N=1 ones-matmul shares its stationary operand with the
            # adjacent ctx0 matmul.
            ps_sum = pst.tile([BC, 1], FP32, tag="sum")
            ps_c0 = psc.tile([BC, 512], FP32, tag="ctx0")
            ps_c1 = psc.tile([BC, 512], FP32, tag="ctx1")
            ctx_sb = cp.tile([BC, D], FP32, tag="ctx_sb")
            # chunk 0 fully first: its scale-copy + DMA-out overlap chunk 1
            for g in range(G):
                s, gl = divmod(g, 4)
                lhs = bd[:, BC * g:BC * (g + 1)]
                nc.tensor.matmul(
                    ps_sum[:], lhs, ones_col[:],
                    start=(g == 0), stop=(g == G - 1),
                )
                nc.tensor.matmul(
                    ps_c0[:], lhs, nat_tiles[s][:, gl * D:gl * D + 512],
                    start=(g == 0), stop=(g == G - 1),
                )
            rec64 = cp.tile([BC, 1], FP32, tag="rec64")
            nc.vector.reciprocal(rec64[:], ps_sum[:])
            nc.vector.tensor_scalar_mul(ctx_sb[:, 0:512], ps_c0[:], rec64[:])
            nc.sync.dma_start(ctx_d[:, 0:512], ctx_sb[:, 0:512])

            # attn-output transpose sits between the two ctx streams so its
            # DVE chain + DMA overlap the ctx1 matmuls.
            pt1 = pst.tile([G, 128], FP32, tag="pt1")
            nc.tensor.transpose(pt1[:], exp_t[:], ident_t[:])

            for g in range(G):
                s, gl = divmod(g, 4)
                nc.tensor.matmul(
                    ps_c1[:], bd[:, BC * g:BC * (g + 1)],
                    nat_tiles[s][:, gl * D + 512:(gl + 1) * D],
                    start=(g == 0), stop=(g == G - 1),
                )
            sums = cp.tile([G, 2], FP32, tag="sums")
            nc.vector.tensor_reduce(
                out=sums[:],
                in_=pt1.rearrange("g (p l) -> g p l", p=2),
                axis=mybir.AxisListType.X,
                op=Alu.add,
            )
            rec = cp.tile([G, 2], FP32, tag="rec")
            nc.vector.reciprocal(rec[:], sums[:])
            attn_sb = cp.tile([G, 128], FP32, tag="attn_sb")
            for p in range(2):
                nc.vector.tensor_scalar_mul(
                    attn_sb[:, p * L:(p + 1) * L],
                    pt1[:, p * L:(p + 1) * L],
                    rec[:, p:p + 1],
                )
            nc.scalar.dma_start(attn_d.rearrange("(g p) l -> g (p l)", p=2), attn_sb[:])

            nc.vector.tensor_scalar_mul(ctx_sb[:, 512:D], ps_c1[:], rec64[:])
            nc.sync.dma_start(ctx_d[:, 512:D], ctx_sb[:, 512:D])

    return nc


def _pack_inputs(features, hidden, W1_w, W1_b, W2_w, W2_b, V_w, V_b):
    features = np.asarray(features, dtype=np.float32)
    hidden = np.asarray(hidden, dtype=np.float32)
    W1_w = np.asarray(W1_w, dtype=np.float32)
    W1_b = np.asarray(W1_b, dtype=np.float32)
    W2_w = np.asarray(W2_w, dtype=np.float32)
    W2_b = np.asarray(W2_b, dtype=np.float32)
    V_w = np.asarray(V_w, dtype=np.float32)

    fb = features.astype(NP_BF16)
    # featT[core, s, dd, (gl, ch, p, l)] = f[core*64 + 8s + 2gl + p, l, 128ch + dd]
    f7 = fb.reshape(NCORES, NS, 4, 2, L, DC, 128)
    featT = np.ascontiguousarray(f7.transpose(0, 1, 6, 2, 5, 3, 4)).reshape(
        NCORES, NS, 128, 4 * D
    )
    # nat[core, s, (p, l), (gl, d)] = f[core*64 + 8s + 2gl + p, l, d]
    f6 = fb.reshape(NCORES, NS, 4, 2, L, D)
    nat = np.ascontiguousarray(f6.transpose(0, 1, 3, 4, 2, 5)).reshape(
        NCORES, NS, 128, 4 * D
    )
    # hT[core, dd, ch, b] = hidden[core*64 + b, 128ch + dd]
    h4 = hidden.reshape(NCORES, BC, UC, 128)
    hT = np.ascontiguousarray(h4.transpose(0, 3, 2, 1)).astype(NP_BF16)

    w1 = np.ascontiguousarray(
        W1_w.astype(NP_BF16).reshape(DC, 128, U).transpose(1, 0, 2)
    )
    w2 = np.ascontiguousarray(
        W2_w.astype(NP_BF16).reshape(UC, 128, U).transpose(1, 0, 2)
    )
    vbc = np.ascontiguousarray(
        np.broadcast_to(V_w.reshape(1, U), (128, U)).astype(NP_BF16)
    )
    ident = np.eye(128, dtype=np.float32)
    # ind64[k, g, m] = 1 iff k == 2g + m//64: the stationary operand that
    # broadcasts g's two rows for group g across the 2*L score rows
    # (zero-padded to K=128 to keep FWL enabled).
    ind64 = np.zeros((128, G, 128), dtype=NP_BF16)
    for g in range(G):
        ind64[2 * g, g, 0:64] = 1
        ind64[2 * g + 1, g, 64:128] = 1
    onesb = np.ones((1, BC), dtype=NP_BF16)
    bsum = (W1_b + W2_b).reshape(1, U).astype(NP_BF16)

    in_maps = []
    for c in range(NCORES):
        in_maps.append(
            {
                "featT": featT[c],
                "nat": nat[c],
                "hT": hT[c],
                "w1": w1,
                "w2": w2,
                "vbc": vbc,
                "ident": ident,
                "ind64": ind64,
                "onesb": onesb,
                "bsum": bsum,
            }
        )
    return in_maps


def _run(inputs, trace=False):
    global _NC_CACHE
    if _NC_CACHE is None:
        _NC_CACHE = _build_program()
    nc = _NC_CACHE
    in_maps = _pack_inputs(**inputs)
    res = run_bass_kernel_spmd(nc, in_maps, list(range(NCORES)), trace=trace)
    ctx = np.concatenate([res.results[c]["ctx"] for c in range(NCORES)], axis=0)
    attn = np.concatenate(
        [res.results[c]["attn"] for c in range(NCORES)], axis=0
    ).reshape(B, L, 1)
    return (ctx, attn), res


def kernel(**inputs):
    out, _ = _run(inputs, trace=False)
    return out


# revision 54
# speedup vs baseline: 1.1945x; 1.1945x over previous
"""Trainium2 Bass kernel for nn_Attention_model_44057774522458.

Bahdanau-style attention:
    h = hidden @ W2 + W2_b                       [B, U]
    score = tanh(features @ W1 + W1_b + h[:,None,:])   [B, L, U]
    logits = score @ V + V_b                     [B, L, 1]
    attn = softmax(logits, axis=1)               [B, L, 1]
    context = sum(attn * features, axis=1)       [B, D]
    returns (context, attn)

B=512, L=64, D=1024, U=512. Data-parallel over B across 8 NeuronCores
(64 batches/core). Per core, batches are processed in 32 groups of 2
(2*L = 128 PE partitions).

Per-group pipeline on each core:
  PE:  score psum [128,512] = sum_c featT[c].T @ W1[c]  (8 bf16 matmuls,
       K=128 each) + indicator.T @ g  (K=2 matmul adds the per-batch
       h+bias correction broadcast over L via a 2-row indicator matrix)
  ACT: tanh psum -> sbuf bf16
  DVE: tensor_tensor_reduce with V broadcast -> logits column [128,1]
Softmax is batched once per core on a [32,128] transpose of the staged
logits ([groups, 2*L] layout: reductions land on the free dim), and
context is a second PE pass: block-diagonal attn columns as stationary
operand against the naturally-laid-out features, accumulating all 32
groups into one [64,512] psum bank per 512-wide D chunk.

features are shipped in two host-packed bf16 layouts (transposed for the
score matmul, natural for the context matmul), 1 MiB contiguous per
4-group superblock per layout.
"""

import numpy as np
import ml_dtypes

import concourse.bass as bass
import concourse.tile as tile
from concourse import mybir
from concourse.bass_utils import run_bass_kernel_spmd

# ---------------------------------------------------------------------------
# Workaround for walrus "Too many sync wait commands" on the TileContext tail
# drain: this neuronxcc build accepts only a single sync-wait on the Drain
# instruction, while Tile attaches one wait per live semaphore. Emit one
# standalone SP wait per (sem, final tick) from the global vector clock
# (validated to reproduce add_sem_waits' wait set exactly), then a wait-free
# drain, then the usual barrier + semaphore clear.
# ---------------------------------------------------------------------------


def _drain_and_barrier_split(self, tick_clock, wait_clock):
    nc = self.nc
    gc = list(tick_clock.global_clock)
    alloc = self.sems.allocated()
    for proc, sem in sorted(alloc.items()):
        tick = gc[proc]
        if tick <= 0:
            continue
        scale = 16 if sem.name.startswith("DMA") else 1
        nc.sync.wait_ge(sem, tick * scale)
    nc.sync.drain()
    nc.all_engine_barrier()
    assert self.sems is not None
    popped = nc._tile_sem_poison_stack.pop()
    assert popped is self._sem_poison
    nc.clear_and_free_semaphores(list(self.sems.allocated().values()))
    nc.all_engine_barrier()


tile.TileContext._drain_and_barrier = _drain_and_barrier_split

# This walrus build also rejects >1 sync-wait on at least the Drain and
# LDWEIGHTS instruction structs ("Too many sync wait commands"). Peel every
# wait beyond the first off onto dedicated InstEventSemaphore instructions
# committed immediately before, on the same engine — engines execute their
# stream in order, so a wait on the preceding instruction is equivalent.
_MAX_WAITS = 1
_orig_commit = tile.TileContext._commit_instruction


def _commit_split_waits(self, inst, lazy_reg_writes=True):
    import bass_rust as _br

    si = inst.sync_info
    if si is not None and len(si.on_wait) > _MAX_WAITS:
        waits = list(si.on_wait)
        keep, extra = waits[:_MAX_WAITS], waits[_MAX_WAITS:]
        for w in extra:
            ev = mybir.InstEventSemaphore(
                name=self.nc.get_next_instruction_name(), ins=[], outs=[]
            )
            ev.engine = inst.engine
            ev.sync_info = _br.SyncInfo(on_wait=[w], on_update=[])
            _orig_commit(self, ev, lazy_reg_writes)
        inst.sync_info = _br.SyncInfo(on_wait=keep, on_update=list(si.on_update))
    return _orig_commit(self, inst, lazy_reg_writes)


tile.TileContext._commit_instruction = _commit_split_waits

BF16 = mybir.dt.bfloat16
FP32 = mybir.dt.float32
FP32R = mybir.dt.float32r  # fp32 bits, full-rate PE mode (1 cyc/row at N>=256)
NP_BF16 = ml_dtypes.bfloat16

B, L, D, U = 512, 64, 1024, 512
NCORES = 8
BC = B // NCORES          # batches per core = 64
G = BC // 2               # 2-batch groups per core = 32
NS = G // 4               # 4-group superblocks per core = 8
DC = D // 128             # 1024/128 = 8 contraction chunks
UC = U // 128             # 512/128 = 4 chunks (h matmul contraction)

_NC_CACHE = None


def _build_program():
    nc = bass.Bass()
    dp = nc.declare_dram_parameter
    featT_d = dp("featT", [NS, 128, 4 * D], BF16, isOutput=False)
    nat_d = dp("nat", [NS, 128, 4 * D], BF16, isOutput=False)
    hT_d = dp("hT", [128, UC, BC], BF16, isOutput=False)
    w1_d = dp("w1", [128, DC, U], BF16, isOutput=False)
    w2_d = dp("w2", [128, UC, U], BF16, isOutput=False)
    vbc_d = dp("vbc", [128, U], BF16, isOutput=False)
    ident_d = dp("ident", [128, 128], FP32, isOutput=False)
    ind_d = dp("ind64", [128, G, 128], BF16, isOutput=False)
    onesb_d = dp("onesb", [1, BC], BF16, isOutput=False)
    bsum_d = dp("bsum", [1, U], BF16, isOutput=False)
    ctx_d = dp("ctx", [BC, D], FP32, isOutput=True)
    attn_d = dp("attn", [BC, L], FP32, isOutput=True)

    Act = mybir.ActivationFunctionType
    Alu = mybir.AluOpType

    with tile.TileContext(nc) as tc:
        with (
            tc.tile_pool(name="consts", bufs=1) as cp,
            tc.tile_pool(name="natp", bufs=1) as natp,
            tc.tile_pool(name="featp", bufs=7) as featp,
            tc.tile_pool(name="work", bufs=3) as wp,
            tc.tile_pool(name="ttrp", bufs=2) as ttrp,
            tc.tile_pool(name="psmain", bufs=4, space="PSUM") as psm,
            tc.tile_pool(name="psctx", bufs=1, space="PSUM") as psc,
            tc.tile_pool(name="pst", bufs=1, space="PSUM") as pst,
        ):
            # ---- loads, ordered by when the PE first needs them ----
            # featT is fetched per group (256 KiB each) with a 6-group
            # prefetch window so the PE never waits; nat loads (needed only
            # in phase D) are spread through the group loop.
            featT_tiles = {}

            def fetch_featT(g):
                s, gl = divmod(g, 4)
                t = featp.tile([128, D], BF16, tag="featT")
                nc.sync.dma_start(t[:], featT_d[s][:, gl * D:(gl + 1) * D])
                featT_tiles[g] = t

            fetch_featT(0)
            w1_t = cp.tile([128, DC, U], BF16, tag="w1")
            nc.sync.dma_start(w1_t[:, 0, :], w1_d[:, 0, :])
            fetch_featT(1)
            fetch_featT(2)
            nc.sync.dma_start(w1_t[:, 1:4, :], w1_d[:, 1:4, :])
            nc.sync.dma_start(w1_t[:, 4:, :], w1_d[:, 4:, :])
            hT_t = cp.tile([128, UC, BC], BF16, tag="hT")
            nc.sync.dma_start(hT_t[:], hT_d[:])
            w2_t = cp.tile([128, UC, U], BF16, tag="w2")
            nc.sync.dma_start(w2_t[:], w2_d[:])
            for g in range(3, 6):
                fetch_featT(g)
            # late-needed consts dispatch in parallel on the Scalar HWDGE ring
            vbc_t = cp.tile([128, U], BF16, tag="vbc")
            nc.scalar.dma_start(vbc_t[:], vbc_d[:])
            ind_t = cp.tile([128, G, 128], BF16, tag="ind64")
            nc.scalar.dma_start(ind_t[:], ind_d[:])
            onesb_t = cp.tile([1, BC], BF16, tag="onesb")
            nc.scalar.dma_start(onesb_t[:], onesb_d[:])
            bsum_t = cp.tile([1, U], BF16, tag="bsum")
            nc.scalar.dma_start(bsum_t[:], bsum_d[:])
            ident_t = cp.tile([128, 128], FP32, tag="ident")
            nc.scalar.dma_start(ident_t[:], ident_d[:])

            zb = cp.tile([128, 1], FP32, tag="zb")
            nc.vector.memset(zb[:], 0.0)
            ones_col = cp.tile([128, 1], BF16, tag="ones_col")
            nc.vector.memset(ones_col[:], 1.0)
            # block-diagonal stationary operand for the context matmuls:
            # column 64*g + (2g+p) holds (unnormalized) exp-attn for batch
            # 2g+p on partitions p*64 .. p*64+64, all else zero.
            bd = cp.tile([128, G * BC], BF16, tag="bd")
            nc.vector.memset(bd[:], 0.0)

            nat_tiles = {}
            # g_bf is K=128 (rows 64-127 zero) so the indicator matmul's
            # weight load stays on the fast (FWL) path.
            g_bf = cp.tile([128, U], BF16, tag="g_bf")
            nc.vector.memset(g_bf[64:128, :], 0.0)
            logits = cp.tile([128, G], FP32, tag="logits")

            # ---- phase B: per-group score -> tanh -> V-dot ----
            # Groups 0-2 emit their feature matmuls first with the indicator
            # (h-broadcast) matmul deferred until after the g-chain, so the
            # PE streams on featT+W1 alone while hT/W2 are still in flight.
            NPRE = 3

            def score_mms(g, with_ind):
                # with_ind: the indicator (h-broadcast) matmul leads the
                # accumulation group so its weight load hides behind the
                # previous group's stream.
                featT_t = featT_tiles.pop(g)
                ps_s = psm.tile([128, U], FP32, tag="score")
                if with_ind:
                    nc.tensor.matmul(
                        ps_s[:], ind_t[:, g, :], g_bf[:], start=True, stop=False
                    )
                for c in range(DC):
                    nc.tensor.matmul(
                        ps_s[:], featT_t[:, c * 128:(c + 1) * 128], w1_t[:, c, :],
                        start=(c == 0 and not with_ind),
                        stop=(with_ind and c == DC - 1),
                    )
                return ps_s

            def finish_group(g, ps_s, emit_ind):
                if emit_ind:
                    nc.tensor.matmul(
                        ps_s[:], ind_t[:, g, :], g_bf[:], start=False, stop=True
                    )
                score_sb = wp.tile([128, U], BF16, tag="score_sb")
                nc.scalar.activation(score_sb[:], ps_s[:], Act.Tanh, bias=zb[:])
                ttr_out = ttrp.tile([128, U], BF16, tag="ttr")
                nc.vector.tensor_mul(ttr_out[:], score_sb[:], vbc_t[:])
                nc.vector.tensor_reduce(
                    out=logits[:, g:g + 1],
                    in_=ttr_out[:],
                    axis=mybir.AxisListType.X,
                    op=Alu.add,
                )

            exp_t = cp.tile([128, G], FP32, tag="exp_t")
            H = G // 2

            def exp_and_bd(half):
                # exp in logits layout + scatter into the block-diagonal
                # operand; context is normalized at the end instead (softmax
                # denominator folded into the psum->sbuf copy), keeping the
                # transpose + attn-output chain off the critical path.
                c0, c1 = half * H, (half + 1) * H
                nc.scalar.activation(
                    exp_t[:, c0:c1], logits[:, c0:c1], Act.Exp, bias=zb[:]
                )
                nc.vector.tensor_copy(
                    bd[0:64, 66 * c0:66 * (c1 - 1) + 1:66], exp_t[0:64, c0:c1]
                )
                nc.vector.tensor_copy(
                    bd[64:128, 66 * c0 + 1:66 * (c1 - 1) + 2:66],
                    exp_t[64:128, c0:c1],
                )

            nt = natp.tile([128, 4 * D], BF16, tag="nat0")
            nc.sync.dma_start(nt[:], nat_d[0])
            nat_tiles[0] = nt

            # prologue groups emit chunk-major so the PE consumes W1 chunks
            # and featT tiles in DMA arrival order without long stalls
            pre_feat = [featT_tiles.pop(g) for g in range(NPRE)]
            pre_psum = [
                psm.tile([128, U], FP32, tag="score", name=f"pre_ps{g}")
                for g in range(NPRE)
            ]
            for c in range(DC):
                for g in range(NPRE):
                    nc.tensor.matmul(
                        pre_psum[g][:],
                        pre_feat[g][:, c * 128:(c + 1) * 128],
                        w1_t[:, c, :],
                        start=(c == 0), stop=False,
                    )
            # g2 = hidden @ W2 + (W2_b + W1_b), fp32r full-rate
            ps_g = psm.tile([BC, U], FP32, tag="score")
            for c in range(UC):
                nc.tensor.matmul(
                    ps_g[:], hT_t[:, c, :], w2_t[:, c, :],
                    start=(c == 0), stop=False,
                )
            nc.tensor.matmul(ps_g[:], onesb_t[:], bsum_t[:], start=False, stop=True)
            nc.vector.tensor_copy(g_bf[0:BC, :], ps_g[:])
            for g in range(NPRE):
                fetch_featT(g + 6)
                finish_group(g, pre_psum[g], emit_ind=True)

            for g in range(NPRE, G):
                s, gl = divmod(g, 4)
                if gl == 0:
                    nt = natp.tile([128, 4 * D], BF16, tag=f"nat{s}")
                    nc.sync.dma_start(nt[:], nat_d[s])
                    nat_tiles[s] = nt
                if g + 6 < G:
                    fetch_featT(g + 6)
                ps_s = score_mms(g, with_ind=True)
                finish_group(g, ps_s, emit_ind=False)
                if g == H - 1:
                    exp_and_bd(0)
            exp_and_bd(1)

            # ---- phase D: context accumulation (unnormalized) ----
            # per-batch softmax denominators ride along in the ctx0 loop:
            # the N=1 ones-matmul shares its stationary operand with the
            # adjacent ctx0 matmul.
            ps_sum = pst.tile([BC, 1], FP32, tag="sum")
            ps_c0 = psc.tile([BC, 512], FP32, tag="ctx0")
            ps_c1 = psc.tile([BC, 512], FP32, tag="ctx1")
            ctx_sb = cp.tile([BC, D], FP32, tag="ctx_sb")
            # chunk 0 fully first: its scale-copy + DMA-out overlap chunk 1
            for g in range(G):
                s, gl = divmod(g, 4)
                lhs = bd[:, BC * g:BC * (g + 1)]
                nc.tensor.matmul(
                    ps_sum[:], lhs, ones_col[:],
                    start=(g == 0), stop=(g == G - 1),
                )
                nc.tensor.matmul(
                    ps_c0[:], lhs, nat_tiles[s][:, gl * D:gl * D + 512],
                    start=(g == 0), stop=(g == G - 1),
                )
            rec64 = cp.tile([BC, 1], FP32, tag="rec64")
            nc.vector.reciprocal(rec64[:], ps_sum[:])
            nc.vector.tensor_scalar_mul(ctx_sb[:, 0:512], ps_c0[:], rec64[:])
            nc.sync.dma_start(ctx_d[:, 0:512], ctx_sb[:, 0:512])

            # attn-output transpose sits between the two ctx streams so its
            # DVE chain + DMA overlap the ctx1 matmuls.
            pt1 = pst.tile([G, 128], FP32, tag="pt1")
            nc.tensor.transpose(pt1[:], exp_t[:], ident_t[:])

            for g in range(G):
                s, gl = divmod(g, 4)
                nc.tensor.matmul(
                    ps_c1[:], bd[:, BC * g:BC * (g + 1)],
                    nat_tiles[s][:, gl * D + 512:(gl + 1) * D],
                    start=(g == 0), stop=(g == G - 1),
                )
            sums = cp.tile([G, 2], FP32, tag="sums")
            nc.vector.tensor_reduce(
                out=sums[:],
                in_=pt1.rearrange("g (p l) -> g p l", p=2),
                axis=mybir.AxisListType.X,
                op=Alu.add,
            )
            rec = cp.tile([G, 2], FP32, tag="rec")
            nc.vector.reciprocal(rec[:], sums[:])
            attn_sb = cp.tile([G, 128], FP32, tag="attn_sb")
            for p in range(2):
                nc.vector.tensor_scalar_mul(
                    attn_sb[:, p * L:(p + 1) * L],
                    pt1[:, p * L:(p + 1) * L],
                    rec[:, p:p + 1],
                )
            nc.scalar.dma_start(attn_d.rearrange("(g p) l -> g (p l)", p=2), attn_sb[:])

            nc.vector.tensor_scalar_mul(ctx_sb[:, 512:D], ps_c1[:], rec64[:])
            nc.sync.dma_start(ctx_d[:, 512:D], ctx_sb[:, 512:D])

    return nc


def _pack_inputs(features, hidden, W1_w, W1_b, W2_w, W2_b, V_w, V_b):
    features = np.asarray(features, dtype=np.float32)
    hidden = np.asarray(hidden, dtype=np.float32)
    W1_w = np.asarray(W1_w, dtype=np.float32)
    W1_b = np.asarray(W1_b, dtype=np.float32)
    W2_w = np.asarray(W2_w, dtype=np.float32)
    W2_b = np.asarray(W2_b, dtype=np.float32)
    V_w = np.asarray(V_w, dtype=np.float32)

    fb = features.astype(NP_BF16)
    # featT[core, s, dd, (gl, ch, p, l)] = f[core*64 + 8s + 2gl + p, l, 128ch + dd]
    f7 = fb.reshape(NCORES, NS, 4, 2, L, DC, 128)
    featT = np.ascontiguousarray(f7.transpose(0, 1, 6, 2, 5, 3, 4)).reshape(
        NCORES, NS, 128, 4 * D
    )
    # nat[core, s, (p, l), (gl, d)] = f[core*64 + 8s + 2gl + p, l, d]
    f6 = fb.reshape(NCORES, NS, 4, 2, L, D)
    nat = np.ascontiguousarray(f6.transpose(0, 1, 3, 4, 2, 5)).reshape(
        NCORES, NS, 128, 4 * D
    )
    # hT[core, dd, ch, b] = hidden[core*64 + b, 128ch + dd]
    h4 = hidden.reshape(NCORES, BC, UC, 128)
    hT = np.ascontiguousarray(h4.transpose(0, 3, 2, 1)).astype(NP_BF16)

    w1 = np.ascontiguousarray(
        W1_w.astype(NP_BF16).reshape(DC, 128, U).transpose(1, 0, 2)
    )
    w2 = np.ascontiguousarray(
        W2_w.astype(NP_BF16).reshape(UC, 128, U).transpose(1, 0, 2)
    )
    vbc = np.ascontiguousarray(
        np.broadcast_to(V_w.reshape(1, U), (128, U)).astype(NP_BF16)
    )
    ident = np.eye(128, dtype=np.float32)
    # ind64[k, g, m] = 1 iff k == 2g + m//64: the stationary operand that
    # broadcasts g's two rows for group g across the 2*L score rows
    # (zero-padded to K=128 to keep FWL enabled).
    ind64 = np.zeros((128, G, 128), dtype=NP_BF16)
    for g in range(G):
        ind64[2 * g, g, 0:64] = 1
        ind64[2 * g + 1, g, 64:128] = 1
    onesb = np.ones((1, BC), dtype=NP_BF16)
    bsum = (W1_b + W2_b).reshape(1, U).astype(NP_BF16)

    in_maps = []
    for c in range(NCORES):
        in_maps.append(
            {
                "featT": featT[c],
                "nat": nat[c],
                "hT": hT[c],
                "w1": w1,
                "w2": w2,
                "vbc": vbc,
                "ident": ident,
                "ind64": ind64,
                "onesb": onesb,
                "bsum": bsum,
            }
        )
    return in_maps


def _run(inputs, trace=False):
    global _NC_CACHE
    if _NC_CACHE is None:
        _NC_CACHE = _build_program()
    nc = _NC_CACHE
    in_maps = _pack_inputs(**inputs)
    res = run_bass_kernel_spmd(nc, in_maps, list(range(NCORES)), trace=trace)
    ctx = np.concatenate([res.results[c]["ctx"] for c in range(NCORES)], axis=0)
    attn = np.concatenate(
        [res.results[c]["attn"] for c in range(NCORES)], axis=0
    ).reshape(B, L, 1)
    return (ctx, attn), res


def kernel(**inputs):
    out, _ = _run(inputs, trace=False)
    return out
